# revision 1
# baseline (speedup 1.0000x reference)
"""BottleneckAttention TRN2 kernel: 8 NeuronCores, one (batch, head) pair per core.

Decomposition (per core, batch b / head i):
  q = (scale * Wq_i) @ x_b          [64, 4096]   (d-major)
  k = Wk_i @ x_b                    [64, 4096]
  vT = (Wv_i @ x_b)^T               [4096, 64]   (n-major, built chunkwise)
  Height rel-bias folded into the score matmul via an augmented contraction:
     K_aug = [k; Ih]  Q_aug = [q; RH^T]     (Ih[h',j] = 1 if j//64 == h')
     S^T[j,q] = K_aug^T Q_aug = content + height-bias
  Width rel-bias applied multiplicatively after exp (separability of exp):
     E = exp(S^T) * ew_dup[jw(j), q],  ew = exp(RW^T)
  PV + row-sums fused: vT_aug = [vT | 1] so out rows 0..63 = unnormalized
  attention output (transposed [d, q]), row 64 = softmax denominators.
  Output projection partial: P = Wout[:, i*64:(i+1)*64] @ out, then columns
  scaled by 1/sums (normalization commutes with the d-contraction).
Host sums the 4 per-head partials per batch and adds the residual x.

Softmax skips the max-subtraction (logits stay within ~[-12, 12]).
"""

import numpy as np

import concourse.bass as bass
import concourse.bacc as bacc
import concourse.tile as tile
from concourse import mybir
from concourse.bass_utils import run_bass_kernel_spmd

F32 = mybir.dt.float32
BF16 = mybir.dt.bfloat16
AF = mybir.ActivationFunctionType

HEADS, B, C, HH, WW = 4, 2, 256, 64, 64
N = HH * WW           # 4096
DH = C // HEADS       # 64
NQ = 4                # query blocks
QB = N // NQ          # 1024 query cols per block
NJC = 32              # key chunks of 128


def _body(tc, io):
    from contextlib import ExitStack
    with ExitStack() as ctx:
        _body_inner(tc, io, ctx)


def _body_inner(tc, io, ctx):
    nc = tc.nc
    xb, wq, wk, wv, wo, relw, relh, ih, out = (
        io["xb"], io["wq"], io["wk"], io["wv"], io["wo"],
        io["relw"], io["relh"], io["ih"], io["out"],
    )

    big = ctx.enter_context(tc.tile_pool(name="big", bufs=1))
    rot = ctx.enter_context(tc.tile_pool(name="rot", bufs=8))
    ep = ctx.enter_context(tc.tile_pool(name="ep", bufs=2))
    spool = ctx.enter_context(tc.tile_pool(name="spool", bufs=2, space="PSUM"))
    opool = ctx.enter_context(tc.tile_pool(name="opool", bufs=2, space="PSUM"))
    dpool = ctx.enter_context(tc.tile_pool(name="dpool", bufs=2, space="DRAM"))

    # ---- input DMAs (weights first; xb split by (cc, half)) --------
    xb_f = big.tile([128, 2, N], F32)
    xv = xb.rearrange("(cc p) n -> p cc n", p=128)
    wq_f = big.tile([128, 2, DH], F32)
    wk_f = big.tile([128, 2, DH], F32)
    wv_f = big.tile([128, 2, DH], F32)
    for t_f, t_d in ((wq_f, wq), (wk_f, wk), (wv_f, wv)):
        nc.sync.dma_start(out=t_f, in_=t_d.rearrange("(cc p) d -> p cc d", p=128))
    wo_f = big.tile([64, 256], F32)
    nc.sync.dma_start(out=wo_f, in_=wo)
    relw_f = big.tile([64, 127], F32)
    nc.sync.dma_start(out=relw_f, in_=relw)
    relh_f = big.tile([64, 127], F32)
    nc.sync.dma_start(out=relh_f, in_=relh)
    ih_f = big.tile([64, N], F32)
    nc.sync.dma_start(out=ih_f, in_=ih)

    wq_bf = big.tile([128, 2, DH], BF16)
    wk_bf = big.tile([128, 2, DH], BF16)
    wv_bf = big.tile([128, 2, DH], BF16)
    for t_bf, t_f in ((wq_bf, wq_f), (wk_bf, wk_f), (wv_bf, wv_f)):
        nc.vector.tensor_copy(out=t_bf, in_=t_f)
    wo_bf = big.tile([64, 256], BF16)
    nc.vector.tensor_copy(out=wo_bf, in_=wo_f)
    relw_bf = big.tile([64, 127], BF16)
    nc.vector.tensor_copy(out=relw_bf, in_=relw_f)
    relh_bf = big.tile([64, 127], BF16)
    nc.vector.tensor_copy(out=relh_bf, in_=relh_f)
    xb_bf = big.tile([128, 2, N], BF16)
    for s in range(4):
        for cc in range(2):
            eng = nc.sync if (s + cc) % 2 == 0 else nc.gpsimd
            eng.dma_start(out=xb_f[:, cc, bass.ts(s, N // 4)],
                          in_=xv[:, cc, bass.ts(s, N // 4)])
            nc.vector.tensor_copy(out=xb_bf[:, cc, bass.ts(s, N // 4)],
                                  in_=xb_f[:, cc, bass.ts(s, N // 4)])

    # PE warm-up: independent dummy matmuls keep the PE HAM busy while
    # the input DMA lands, so real matmuls start at the warm clock.
    warm = big.tile([128, 512], BF16)
    nc.vector.memset(warm, 0.0)
    wps = spool.tile([128, 512], F32, tag="sp")
    for _ in range(48):
        nc.tensor.matmul(wps, warm[:, 0:128], warm, start=True, stop=True)

    K_aug = big.tile([128, N], BF16)
    Q_aug = big.tile([128, N], BF16)
    ew_dup = big.tile([128, N], BF16)
    rwt = big.tile([64, N], BF16)
    vt_aug = big.tile([128, NJC, 65], BF16)
    h_sb = big.tile([64, N], BF16)

    # Ih rows of K_aug (f32 -> bf16 cast)
    nc.scalar.activation(out=K_aug[64:128, :], in_=ih_f, func=AF.Copy)

    def qk_build(dst, w_bf, qq):
        ps = spool.tile([128, QB], F32, tag="sp")
        for cc in range(2):
            for h in range(2):
                nc.tensor.matmul(
                    ps[0:64, bass.ts(h, 512)],
                    w_bf[:, cc, :],
                    xb_bf[:, cc, qq * QB + h * 512: qq * QB + (h + 1) * 512],
                    start=(cc == 0), stop=(cc == 1),
                )
        nc.scalar.activation(out=dst[0:64, bass.ts(qq, QB)],
                             in_=ps[0:64, :], func=AF.Copy)

    q_xy = Q_aug[0:64, :].rearrange("d (x y) -> d x y", y=64)
    rwt_xy = rwt.rearrange("jw (x y) -> jw x y", y=64)

    def rh_build(g):
        # RH^T[jh, n=(x,y)] = sum_d relh[jh - x + 63, d] * q[d, n]
        ps = spool.tile([128, QB], F32, tag="sp")
        for xi in range(16):
            xx = g * 16 + xi
            nc.tensor.matmul(
                ps[0:64, bass.ts(xi, 64)],
                relh_bf[:, 63 - xx: 127 - xx],
                Q_aug[0:64, xx * 64: (xx + 1) * 64],
                start=True, stop=True,
            )
        nc.scalar.activation(out=Q_aug[64:128, bass.ts(g, QB)],
                             in_=ps[0:64, :], func=AF.Copy)

    def rw_build(g):
        # RW^T[jw, n=(x,y)] = sum_d relw[jw - y + 63, d] * q[d, n]
        ps = spool.tile([128, QB], F32, tag="sp")
        for yi in range(16):
            yy = g * 16 + yi
            nc.tensor.matmul(
                ps[0:64, bass.ts(yi, 64)],
                relw_bf[:, 63 - yy: 127 - yy],
                q_xy[:, :, yy],
                start=True, stop=True,
            )
        # ps free layout is [yi, x]; rwt quarter slice wants [x, y].
        # One strided copy with a transposed view of the psum tile.
        nc.vector.tensor_copy(
            out=rwt_xy[:, :, g * 16:(g + 1) * 16],
            in_=ps[0:64, :].rearrange("p (yi x) -> p x yi", x=64))

    def vt_build(g):
        ps = spool.tile([128, 8, 64], F32, tag="sp")
        for ci in range(8):
            chunk = g * 8 + ci
            for cc in range(2):
                nc.tensor.matmul(
                    ps[:, ci, :],
                    xb_bf[:, cc, chunk * 128: (chunk + 1) * 128],
                    wv_bf[:, cc, :],
                    start=(cc == 0), stop=(cc == 1),
                )
        nc.scalar.activation(out=vt_aug[:, g * 8: (g + 1) * 8, 0:64],
                             in_=ps, func=AF.Copy)

    # Pre-main builds: only what quarter 0 needs up-front. q feeds RW
    # (the ew wall), so q and RW come first; the remaining k/vT/RH
    # groups are injected into quarter 0's stream below.
    nc.vector.memset(vt_aug[:, :, 64:65], 1.0)
    for qq in range(NQ):
        qk_build(Q_aug, wq_bf, qq)
    for g in range(4):
        rw_build(g)
    nc.scalar.activation(out=ew_dup[0:64, :], in_=rwt, func=AF.Exp)
    nc.vector.tensor_copy(out=ew_dup[64:128, :], in_=ew_dup[0:64, :])
    rh_build(0)
    qk_build(K_aug, wk_bf, 0)
    vt_build(0)

    # ---- main attention loop ---------------------------------------
    # Per query block: S^T matmul (PE) -> exp (ACT) -> *ew (DVE) -> PV (PE).
    # PV emission lags S by PVLAG stages so short DVE stalls (reciprocal,
    # copies) never stall the in-order PE stream. The previous block's
    # epilogue is spread across this block's stream in small pieces.
    def rh_build_v(g):
        ps = spool.tile([128, QB], F32, tag="sp")
        for xi in range(16):
            xx = g * 16 + xi
            nc.tensor.matmul(
                ps[0:64, bass.ts(xi, 64)],
                relh_bf[:, 63 - xx: 127 - xx],
                Q_aug[0:64, xx * 64: (xx + 1) * 64],
                start=True, stop=True,
            )
        nc.vector.tensor_copy(out=Q_aug[64:128, bass.ts(g, QB)],
                              in_=ps[0:64, :])

    def qk_build_v(dst, w_bf, qq):
        ps = spool.tile([128, QB], F32, tag="sp")
        for cc in range(2):
            for h in range(2):
                nc.tensor.matmul(
                    ps[0:64, bass.ts(h, 512)],
                    w_bf[:, cc, :],
                    xb_bf[:, cc, qq * QB + h * 512: qq * QB + (h + 1) * 512],
                    start=(cc == 0), stop=(cc == 1),
                )
        nc.vector.tensor_copy(out=dst[0:64, bass.ts(qq, QB)],
                              in_=ps[0:64, :])

    def vt_build_v(g):
        ps = spool.tile([128, 8, 64], F32, tag="sp")
        for ci in range(8):
            chunk = g * 8 + ci
            for cc in range(2):
                nc.tensor.matmul(
                    ps[:, ci, :],
                    xb_bf[:, cc, chunk * 128: (chunk + 1) * 128],
                    wv_bf[:, cc, :],
                    start=(cc == 0), stop=(cc == 1),
                )
        nc.vector.tensor_copy(out=vt_aug[:, g * 8: (g + 1) * 8, 0:64], in_=ps)

    PVLAG = 4
    part1 = [None] * NQ             # per-quarter deferred epilogue pieces
    part2 = [None] * NQ

    def make_part1(qq, o_ps):
        rsb = ep.tile([128, QB], F32, tag="rsb")
        rdram = dpool.tile([1, QB], F32, tag="rd")
        rbc = ep.tile([128, QB], F32, tag="rbc")

        def recip_half(h):
            nc.vector.reciprocal(out=rsb[64:65, bass.ts(h, 512)],
                                 in_=o_ps[64:65, bass.ts(h, 512)])

        def bcast():
            nc.sync.dma_start(out=rdram, in_=rsb[64:65, :])
            nc.sync.dma_start(
                out=rbc,
                in_=bass.AP(tensor=rdram.tensor, offset=rdram.offset,
                            ap=[[0, 128]] + list(rdram.ap[1:])),
            )

        def h_copy():
            nc.vector.tensor_copy(out=h_sb[:, bass.ts(qq, QB)],
                                  in_=o_ps[0:64, :])

        return [lambda: recip_half(0), lambda: recip_half(1), bcast, h_copy], rbc

    def make_part2(qq, rbc):
        def proj():
            for oh in range(2):
                pp = spool.tile([128, QB], F32, tag="sp")
                for h in range(2):
                    nc.tensor.matmul(
                        pp[:, bass.ts(h, 512)],
                        wo_bf[:, oh * 128: (oh + 1) * 128],
                        h_sb[:, qq * QB + h * 512: qq * QB + (h + 1) * 512],
                        start=True, stop=True,
                    )
                osb = ep.tile([128, QB], F32, tag="osb")
                nc.vector.tensor_mul(osb, pp, rbc)
                nc.sync.dma_start(
                    out=out[oh * 128: (oh + 1) * 128, qq * QB: (qq + 1) * QB],
                    in_=osb,
                )
        return proj

    for qq in range(NQ):
        o_ps = opool.tile([128, QB], F32)
        e_tiles = [None] * NJC

        def s_stage(jc):
            ps = spool.tile([128, QB], F32, tag="sp")
            for h in range(2):
                nc.tensor.matmul(
                    ps[:, bass.ts(h, 512)],
                    K_aug[:, jc * 128: (jc + 1) * 128],
                    Q_aug[:, qq * QB + h * 512: qq * QB + (h + 1) * 512],
                    start=True, stop=True,
                )
            e0 = rot.tile([128, QB], BF16, tag="e0")
            nc.scalar.activation(out=e0, in_=ps, func=AF.Exp)
            e = rot.tile([128, QB], BF16, tag="e")
            nc.vector.tensor_mul(e, e0, ew_dup[:, bass.ts(qq, QB)])
            e_tiles[jc] = e

        def pv_stage(jc):
            for h in range(2):
                nc.tensor.matmul(
                    o_ps[0:65, bass.ts(h, 512)],
                    vt_aug[:, jc, :],
                    e_tiles[jc][:, bass.ts(h, 512)],
                    start=(jc == 0), stop=(jc == NJC - 1),
                )
            e_tiles[jc] = None

        prev = part1[qq - 1][0] if qq > 0 else None
        for t in range(NJC + PVLAG):
            if t < NJC:
                s_stage(t)
            if qq == 0:
                if t == 2:
                    qk_build_v(K_aug, wk_bf, 1)
                elif t == 4:
                    vt_build_v(1)
                elif t == 8:
                    qk_build_v(K_aug, wk_bf, 2)
                elif t == 10:
                    vt_build_v(2)
                elif t == 14:
                    qk_build_v(K_aug, wk_bf, 3)
                elif t == 16:
                    vt_build_v(3)
                elif t == 20:
                    rh_build_v(1)
                elif t == 24:
                    rh_build_v(2)
                elif t == 28:
                    rh_build_v(3)
            else:
                if t == 6:
                    prev[0]()       # recip half 0
                elif t == 10:
                    prev[1]()       # recip half 1
                elif t == 12:
                    prev[2]()       # broadcast DMAs
                elif t == 14:
                    prev[3]()       # h copy
                elif t == 24:
                    part2[qq - 1]()
            if t >= PVLAG:
                pv_stage(t - PVLAG)

        pieces, rbc = make_part1(qq, o_ps)
        part1[qq] = (pieces,)
        part2[qq] = make_part2(qq, rbc)

    # final quarter epilogue at the tail
    for fn in part1[NQ - 1][0]:
        fn()
    part2[NQ - 1]()


_NC_CACHE = {}


def _build():
    if "nc" in _NC_CACHE:
        return _NC_CACHE["nc"]
    nc = bacc.Bacc("TRN2", target_bir_lowering=False, debug=False, num_devices=8)
    io = {
        "xb": nc.dram_tensor("xb", [C, N], F32, kind="ExternalInput").ap(),
        "wq": nc.dram_tensor("wq", [C, DH], F32, kind="ExternalInput").ap(),
        "wk": nc.dram_tensor("wk", [C, DH], F32, kind="ExternalInput").ap(),
        "wv": nc.dram_tensor("wv", [C, DH], F32, kind="ExternalInput").ap(),
        "wo": nc.dram_tensor("wo", [DH, C], F32, kind="ExternalInput").ap(),
        "relw": nc.dram_tensor("relw", [DH, 127], F32, kind="ExternalInput").ap(),
        "relh": nc.dram_tensor("relh", [DH, 127], F32, kind="ExternalInput").ap(),
        "ih": nc.dram_tensor("ih", [64, N], F32, kind="ExternalInput").ap(),
        "out": nc.dram_tensor("out", [C, N], F32, kind="ExternalOutput").ap(),
    }
    with tile.TileContext(nc) as tc:
        _body(tc, io)
    nc.compile()
    _NC_CACHE["nc"] = nc
    return nc


_last_in_maps = None


def kernel(x, w_qkv, w_out, rel_height, rel_width):
    global _last_in_maps
    x = np.ascontiguousarray(x, np.float32)
    w_qkv = np.asarray(w_qkv, np.float32)
    w_out = np.asarray(w_out, np.float32)
    rel_height = np.asarray(rel_height, np.float32)
    rel_width = np.asarray(rel_width, np.float32)

    scale = np.float32(DH ** -0.5)
    ih_const = np.repeat(np.eye(64, dtype=np.float32), 64, axis=1)
    relw_t = np.ascontiguousarray(rel_width.T)
    relh_t = np.ascontiguousarray(rel_height.T)

    in_maps = []
    for g in range(8):
        b, i = divmod(g, HEADS)
        sl = slice(i * DH, (i + 1) * DH)
        in_maps.append({
            "xb": np.ascontiguousarray(x[b].reshape(C, N)),
            "wq": np.ascontiguousarray((w_qkv[i * DH:(i + 1) * DH] * scale).T),
            "wk": np.ascontiguousarray(w_qkv[C + i * DH: C + (i + 1) * DH].T),
            "wv": np.ascontiguousarray(w_qkv[2 * C + i * DH: 2 * C + (i + 1) * DH].T),
            "wo": np.ascontiguousarray(w_out[:, sl].T),
            "relw": relw_t,
            "relh": relh_t,
            "ih": ih_const,
        })

    _last_in_maps = in_maps
    nc = _build()
    res = run_bass_kernel_spmd(nc, in_maps, core_ids=list(range(8)))
    parts = [r["out"] for r in res.results]
    out = np.empty((B, C, N), np.float32)
    for b in range(B):
        out[b] = parts[4 * b] + parts[4 * b + 1] + parts[4 * b + 2] + parts[4 * b + 3]
        out[b] += x[b].reshape(C, N)
    return out.reshape(B, C, HH, WW)

